# revision 6
# baseline (speedup 1.0000x reference)
import sys

for p in ("/opt/trn_rl_repo",):
    if p not in sys.path:
        sys.path.insert(0, p)

import numpy as np

import concourse.bass as bass
import concourse.bacc as bacc_mod
import concourse.mybir as mybir
from concourse.tile import TileContext
from concourse.masks import make_identity
from concourse.bass_utils import run_bass_kernel_spmd
from concourse.bass import ds

B, T, C, HS = 1024, 128, 384, 64
NCORES = 8
BPC = B // NCORES          # 128 batches per core
NB = 4                     # batches per group (packed along PSUM free dim)
NG = BPC // NB             # 32 groups per core
CK = C // 128              # 3 contraction chunks

_DT = mybir.dt.float32


def build_nc():
    nc = bacc_mod.Bacc(target_bir_lowering=False)

    # x per core, host-prepped layout [group, C, j, T] so each SBUF partition
    # reads contiguous 2KB bursts
    x_d = nc.dram_tensor("x", [NG, C, NB, T], _DT, kind="ExternalInput")
    # Wq|Wk concatenated along output dim, chunked over C: [p, c, m]
    wqk_d = nc.dram_tensor("wqk", [128, CK, 128], _DT, kind="ExternalInput")
    wv_d = nc.dram_tensor("wv", [128, CK, HS], _DT, kind="ExternalInput")
    out_d = nc.dram_tensor("out", [NG, T, NB, HS], _DT, kind="ExternalOutput")

    with TileContext(nc) as tc:
        with (
            tc.tile_pool(name="const", bufs=1) as cpool,
            tc.tile_pool(name="sb", bufs=3) as sbp,
            tc.tile_pool(name="ps2", bufs=2, space="PSUM") as psp2,
            tc.tile_pool(name="ps1", bufs=1, space="PSUM") as psp1,
        ):
            ident = cpool.tile([128, 128], _DT, tag="ident")
            make_identity(nc, ident)
            wqk = cpool.tile([128, CK, 128], _DT, tag="wqk")
            nc.sync.dma_start(out=wqk, in_=wqk_d[:])
            wv = cpool.tile([128, CK, HS], _DT, tag="wv")
            nc.sync.dma_start(out=wv, in_=wv_d[:])

            for g in range(NG):
                xt = sbp.tile([128, CK, NB * T], _DT, tag="xt")
                nc.sync.dma_start(
                    out=xt, in_=x_d[g].rearrange("(c p) j t -> p c (j t)", p=128)
                )

                # q^T / k^T [h, (j t)] for all NB batches per accumulation group
                qT_ps = psp1.tile([64, NB * T], _DT, tag="qT_ps")
                kT_ps = psp1.tile([64, NB * T], _DT, tag="kT_ps")
                for c in range(CK):
                    nc.tensor.matmul(
                        qT_ps, wqk[:, c, 0:64], xt[:, c],
                        start=(c == 0), stop=(c == CK - 1),
                    )
                for c in range(CK):
                    nc.tensor.matmul(
                        kT_ps, wqk[:, c, 64:128], xt[:, c],
                        start=(c == 0), stop=(c == CK - 1),
                    )
                qT = sbp.tile([64, NB * T], _DT, tag="qT")
                nc.vector.tensor_copy(qT, qT_ps)
                kT = sbp.tile([64, NB * T], _DT, tag="kT")
                nc.vector.tensor_copy(kT, kT_ps)

                # v in natural [s, h] layout per batch
                v_ps = psp2.tile([128, NB, HS], _DT, tag="v_ps")
                for j in range(NB):
                    for c in range(CK):
                        nc.tensor.matmul(
                            v_ps[:, j],
                            xt[:, c, ds(j * T, T)],
                            wv[:, c],
                            start=(c == 0),
                            stop=(c == CK - 1),
                        )
                v_sb = sbp.tile([128, NB, HS], _DT, tag="v_sb")
                nc.vector.tensor_copy(v_sb, v_ps)

                # raw scores q @ k^T  (scale folded into exp below)
                wei_ps = psp2.tile([128, NB, T], _DT, tag="wei_ps")
                for j in range(NB):
                    nc.tensor.matmul(
                        wei_ps[:, j],
                        qT[:, ds(j * T, T)],
                        kT[:, ds(j * T, T)],
                        start=True,
                        stop=True,
                    )

                # softmax over s (free axis). Row max over the FULL row (incl.
                # future positions) is a valid shift; masked cols are zeroed
                # post-exp before the sum.
                negmax = sbp.tile([128, NB], _DT, tag="negmax")
                nc.vector.tensor_reduce(
                    negmax, wei_ps, axis=mybir.AxisListType.X,
                    op=mybir.AluOpType.max, negate=True,
                )
                wei_n = sbp.tile([128, NB, T], _DT, tag="wei_n")
                nc.vector.tensor_tensor(
                    wei_n, wei_ps,
                    negmax[:, :, None].to_broadcast((128, NB, T)),
                    mybir.AluOpType.add,
                )
                p_sb = sbp.tile([128, NB, T], _DT, tag="p_sb")
                nc.scalar.activation(
                    out=p_sb, in_=wei_n,
                    func=mybir.ActivationFunctionType.Exp,
                    scale=0.125,
                )
                # causal: keep s <= t (partition index), zero the rest
                nc.gpsimd.affine_select(
                    out=p_sb, in_=p_sb,
                    compare_op=mybir.AluOpType.is_ge,
                    fill=0.0, base=0,
                    pattern=[[0, NB], [-1, T]],
                    channel_multiplier=1,
                )
                rowsum = sbp.tile([128, NB], _DT, tag="rowsum")
                nc.vector.tensor_reduce(
                    rowsum, p_sb, axis=mybir.AxisListType.X, op=mybir.AluOpType.add
                )
                recip = sbp.tile([128, NB], _DT, tag="recip")
                nc.vector.reciprocal(recip, rowsum)

                pT_ps = psp1.tile([128, NB, T], _DT, tag="pT_ps")
                for j in range(NB):
                    nc.tensor.transpose(pT_ps[:, j], p_sb[:, j], ident)
                pT_sb = sbp.tile([128, NB, T], _DT, tag="pT_sb")
                nc.vector.tensor_copy(pT_sb, pT_ps)

                out_ps = psp1.tile([128, NB, HS], _DT, tag="out_ps")
                for j in range(NB):
                    nc.tensor.matmul(
                        out_ps[:, j], pT_sb[:, j], v_sb[:, j], start=True, stop=True
                    )
                out_sb = sbp.tile([128, NB, HS], _DT, tag="out_sb")
                nc.vector.tensor_tensor(
                    out_sb, out_ps,
                    recip[:, :, None].to_broadcast((128, NB, HS)),
                    mybir.AluOpType.mult,
                )
                nc.sync.dma_start(out=out_d[g], in_=out_sb)

    nc.finalize()
    return nc


_NC_CACHE = None


def kernel(x, Wq, Wk, Wv):
    global _NC_CACHE
    x = np.asarray(x, dtype=np.float32)
    # [B,T,C] -> [core, g, C, j, t] with j (batch-within-group) inner so DMA
    # bursts are 2KB contiguous per partition
    xp = np.ascontiguousarray(
        x.reshape(NCORES, NG, NB, T, C).transpose(0, 1, 4, 2, 3)
    )
    wqk = np.ascontiguousarray(
        np.concatenate([Wq, Wk], axis=1).reshape(CK, 128, 128).transpose(1, 0, 2),
        dtype=np.float32,
    )
    wvp = np.ascontiguousarray(
        np.asarray(Wv, dtype=np.float32).reshape(CK, 128, HS).transpose(1, 0, 2)
    )
    if _NC_CACHE is None:
        _NC_CACHE = build_nc()
    nc = _NC_CACHE
    in_maps = [{"x": xp[i], "wqk": wqk, "wv": wvp} for i in range(NCORES)]
    res = run_bass_kernel_spmd(nc, in_maps, core_ids=list(range(NCORES)))
    outs = np.stack([res.results[i]["out"] for i in range(NCORES)])
    # [core, g, T, j, HS] -> [B, T, HS]
    return np.ascontiguousarray(
        outs.transpose(0, 1, 3, 2, 4).reshape(B, T, HS)
    )
